# revision 28
# baseline (speedup 1.0000x reference)
"""Causal single-head attention (B=4, S=2048, D=1024) on 8 TRN2 NeuronCores.

Sharding: 2 cores per batch; each core owns 8 q-blocks of 128 rows chosen so
both cores of a batch see the same multiset of causal kv-span lengths
(padded to 512-chunks): core h=0 -> q-blocks [0,3,4,7,8,11,12,15],
core h=1 -> [1,2,5,6,9,10,13,14]; both give span chunks [1,1,2,2,3,3,4,4].
This makes one SPMD program valid for all 8 cores; per-core differences
(which q rows, causal mask offsets) ride in the input data.

Math per core (all matmuls in float32r, fp32 accumulation), with the host
folding M = Wq @ Wk^T / sqrt(D) so no K-projection is needed on device:
  A^T = M^T @ qT                                      (single projection)
  S_i = A_i^T.T @ kT (+ additive causal mask)         (scores vs RAW k^T)
  P = exp(S), denom = rowsum(P)                       (no max-sub: |S| < ~10)
  T_i = (P @ v) / denom                               (reassociated: raw v!)
  out_i = T_i @ Wv                                    (deferred out-proj)
Reassociation (P@v)@Wv replaces attn@(v@Wv) - saves the V projection.
"""

import os
from contextlib import ExitStack

import ml_dtypes
import numpy as np

import concourse.bass as bass
import concourse.mybir as mybir
import concourse.tile as tile
from concourse import bacc
from concourse.bass_utils import run_bass_kernel_spmd

B, S, D = 4, 2048, 1024
P = 128                      # partitions / q-block rows
NBLK = 8                     # q-blocks per core
CH = 512                     # kv chunk (matmul moving free dim)
# computed kv width per q-block position: max causal span over the two cores
# of a pair (so the program stays uniform), rounded up to 128
W = [256, 512, 768, 1024, 1280, 1536, 1792, 2048]
ORDER_A = [0, 2, 4, 5, 6, 7]    # first segment: needs all of v
ORDER_B = [3, 1]                # tail segment: only v chunks 0-7; frees SBUF
                                # so phase-4 inputs (wv, tt) prefetch under it
BLOCKS = [[0, 3, 4, 7, 8, 11, 12, 15], [1, 2, 5, 6, 9, 10, 13, 14]]
MASK_BASE = [[0, 384], [128, 256]]   # base[h][pos % 2]: col c allowed iff c <= base + r
DT = mybir.dt.float32r
F32 = mybir.dt.float32
NEG = -1e30

_cached = {}


def _build():
    if "nc" in _cached:
        return _cached["nc"]
    nc = bacc.Bacc("TRN2", target_bir_lowering=False, debug=False, num_devices=8)
    qT = nc.dram_tensor("qT", [D, P * NBLK], DT, kind="ExternalInput").ap()
    kT = nc.dram_tensor("kT", [D, S], DT, kind="ExternalInput").ap()
    v = nc.dram_tensor("v", [S, D], DT, kind="ExternalInput").ap()
    wq = nc.dram_tensor("wq", [D, D], DT, kind="ExternalInput").ap()
    wv = nc.dram_tensor("wv", [D, D], DT, kind="ExternalInput").ap()
    mask = nc.dram_tensor("mask", [P, NBLK, CH], mybir.dt.bfloat16,
                          kind="ExternalInput").ap()
    ident = nc.dram_tensor("ident", [P, P], DT, kind="ExternalInput").ap()
    out = nc.dram_tensor("out", [P * NBLK, D], F32, kind="ExternalOutput").ap()

    KO = D // P      # 8 contraction chunks
    NV = S // P      # 16 v row-chunks
    QK = S // 4      # kv quarter for kT staging

    kT_r = kT.rearrange("(ko p) s -> p ko s", p=P)
    v_r = v.rearrange("(so p) d -> p so d", p=P)
    wv_r = wv.rearrange("(ko p) m -> p ko m", p=P)

    with tile.TileContext(nc) as tc:
        with tc.tile_pool(name="pers", bufs=1) as pers, \
             tc.tile_pool(name="dram", bufs=1, space="DRAM") as dpool:
            ident_sb = pers.tile([P, P], DT)
            nc.sync.dma_start(ident_sb[:], ident)
            warm = pers.tile([P, 1], F32)
            nc.scalar.activation(warm[:], ident_sb[:, 0:1].bitcast(F32),
                                 mybir.ActivationFunctionType.Exp)
            mask_sb = pers.tile([P, NBLK, CH], mybir.dt.bfloat16)
            nc.sync.dma_start(mask_sb[:], mask)
            QT_sb = pers.tile([P, KO, P * NBLK], DT)
            KT_sb = pers.tile([P, KO, S], DT)
            tt_dram = [dpool.tile([P, D], DT, name=f"ttd_{i}") for i in range(NBLK)]

            # ---- Phase 1: A-projection (A^T = M^T qT, M folded on host);
            #      raw k^T and v-lo stream into SBUF underneath it ----
            _vstack = ExitStack()
            vlo_pool = _vstack.enter_context(tc.tile_pool(name="vlo", bufs=1))
            v_lo = vlo_pool.tile([P, NV // 2, D], DT)
            with tc.tile_pool(name="ps_proj", bufs=4, space="PSUM") as psp:
                with tc.tile_pool(name="qproj", bufs=1) as qpool:
                    qT_sb = qpool.tile([P, KO, P * NBLK], DT)
                    wq_sb = qpool.tile([P, KO, D], DT)
                    wq_r2 = wq.rearrange("(ko p) m -> p ko m", p=P)
                    qT_r2 = qT.rearrange("(ko p) s -> p ko s", p=P)
                    for ko in range(KO):
                        nc.sync.dma_start(wq_sb[:, ko, 0:D // 2],
                                          wq_r2[:, ko, 0:D // 2])
                    for ko in range(KO):
                        nc.sync.dma_start(qT_sb[:, ko, 0:CH], qT_r2[:, ko, 0:CH])
                    for ko in range(KO):
                        nc.sync.dma_start(wq_sb[:, ko, D // 2:D],
                                          wq_r2[:, ko, D // 2:D])
                    for ko in range(KO):
                        nc.sync.dma_start(qT_sb[:, ko, CH:P * NBLK],
                                          qT_r2[:, ko, CH:P * NBLK])
                    for ko in range(KO):
                        nc.sync.dma_start(KT_sb[:, ko, 0:CH], kT_r[:, ko, 0:CH])
                    for so in range(NV // 4):
                        nc.sync.dma_start(v_lo[:, so], v_r[:, so])
                    for mh, n in ((0, 0), (1, 0), (0, 1), (1, 1)):
                        for m in range(mh * 4, mh * 4 + 4):
                            ps = psp.tile([P, CH], F32, tag="pp")
                            for k in range(KO):
                                nc.tensor.matmul(
                                    ps[:], wq_sb[:, k, bass.ts(m, P)],
                                    qT_sb[:, k, bass.ts(n, CH)],
                                    start=(k == 0), stop=(k == KO - 1))
                            nc.vector.tensor_copy(QT_sb[:, m, bass.ts(n, CH)], ps[:])
                        if (mh, n) == (1, 0):
                            for sc in range(1, S // CH):
                                for ko in range(KO):
                                    nc.sync.dma_start(
                                        KT_sb[:, ko, bass.ts(sc, CH)],
                                        kT_r[:, ko, bass.ts(sc, CH)])
                            for so in range(NV // 4, NV // 2):
                                nc.sync.dma_start(v_lo[:, so], v_r[:, so])

            # ---- Phase 3: attention per q-block; T spilled to DRAM.
            #      Pipelined: scores run one chunk ahead of transpose+AV. ----
            if True:
                with tc.tile_pool(name="cwork", bufs=2) as cwork, \
                     tc.tile_pool(name="ppool", bufs=3) as ppool, \
                     tc.tile_pool(name="ptpool", bufs=4) as ptpool, \
                     tc.tile_pool(name="ps_s", bufs=4, space="PSUM") as ps_s, \
                     tc.tile_pool(name="ps_tr", bufs=2, space="PSUM") as ps_tr, \
                     tc.tile_pool(name="ps_t", bufs=1, space="PSUM") as ps_t:
                    def v_chunk(kvi):
                        if kvi < NV // 2:
                            return v_lo[:, kvi]
                        return v_hi[:, kvi - NV // 2]

                    def attention_block(i):
                        wi = W[i]
                        nch = (wi + CH - 1) // CH
                        nkv = wi // P
                        ps_T0 = ps_t.tile([P, CH], F32, tag="T0",
                                          name=f"T0_{i}")
                        ps_T1 = ps_t.tile([P, CH], F32, tag="T1",
                                          name=f"T1_{i}")
                        dsums = []
                        p_tiles = []

                        def emit_scores(c, i=i, nch=nch, wi=wi):
                            w = min(CH, wi - c * CH)
                            ps_c = ps_s.tile([P, CH], F32, tag="s",
                                             name=f"s_{i}_{c}")
                            for k in range(KO):
                                nc.tensor.matmul(
                                    ps_c[:, 0:w], QT_sb[:, k, bass.ts(i, P)],
                                    KT_sb[:, k, bass.ds(c * CH, w)],
                                    start=(k == 0), stop=(k == KO - 1))
                            if c == nch - 1:
                                nc.vector.tensor_tensor(
                                    ps_c[:, 0:w], ps_c[:, 0:w],
                                    mask_sb[:, i, 0:w], mybir.AluOpType.add)
                            p_sb = ppool.tile([P, CH], DT, tag="p",
                                              name=f"p_{i}_{c}")
                            ds = cwork.tile([P, 1], F32, tag="ds",
                                            name=f"ds_{i}_{c}")
                            nc.scalar.activation(
                                p_sb[:, 0:w], ps_c[:, 0:w],
                                mybir.ActivationFunctionType.Exp, accum_out=ds[:])
                            dsums.append(ds)
                            p_tiles.append(p_sb)

                        def emit_trav(c, i=i, nkv=nkv, wi=wi):
                            # transposes run 2 ahead of the AV matmuls
                            nt = min(CH, wi - c * CH) // P
                            pts = []
                            for t in range(nt):
                                ptr = ps_tr.tile([P, P], DT, tag="tr")
                                nc.tensor.transpose(
                                    ptr[:], p_tiles[c][:, bass.ts(t, P)],
                                    ident_sb[:])
                                pt_sb = ptpool.tile([P, P], DT, tag="pt")
                                nc.vector.tensor_copy(pt_sb[:], ptr[:])
                                pts.append(pt_sb)
                                if t >= 2:
                                    _emit_av(c, t - 2, pts[t - 2], i, nkv)
                            for t in range(max(0, nt - 2), nt):
                                _emit_av(c, t, pts[t], i, nkv)

                        def _emit_av(c, t, pt_sb, i, nkv):
                            kvi = c * (CH // P) + t
                            vc = v_chunk(kvi)
                            nc.tensor.matmul(
                                ps_T0[:], pt_sb[:], vc[:, 0:CH],
                                start=(kvi == 0), stop=(kvi == nkv - 1))
                            nc.tensor.matmul(
                                ps_T1[:], pt_sb[:], vc[:, CH:D],
                                start=(kvi == 0), stop=(kvi == nkv - 1))

                        for c in range(nch):
                            emit_scores(c)
                            if c >= 1:
                                emit_trav(c - 1)
                        emit_trav(nch - 1)

                        denom = cwork.tile([P, 1], F32, tag="den")
                        if nch == 1:
                            nc.vector.tensor_copy(denom[:], dsums[0][:])
                        else:
                            nc.vector.tensor_tensor(
                                denom[:], dsums[0][:], dsums[1][:],
                                mybir.AluOpType.add)
                            for c in range(2, nch):
                                nc.vector.tensor_tensor(
                                    denom[:], denom[:], dsums[c][:],
                                    mybir.AluOpType.add)
                        rden = cwork.tile([P, 1], F32, tag="rden")
                        nc.vector.reciprocal(rden[:], denom[:])
                        t_st = cwork.tile([P, D], DT, tag="tst", bufs=1)
                        nc.vector.tensor_scalar_mul(t_st[:, 0:CH], ps_T0[:], rden[:])
                        nc.vector.tensor_scalar_mul(t_st[:, CH:D], ps_T1[:], rden[:])
                        tt_st = cwork.tile([P, KO, P], DT, tag="ttst", bufs=1)
                        for d in range(KO):
                            ptr = ps_tr.tile([P, P], DT, tag="tr")
                            nc.tensor.transpose(
                                ptr[:], t_st[:, bass.ts(d, P)], ident_sb[:])
                            nc.vector.tensor_copy(tt_st[:, d], ptr[:])
                        nc.sync.dma_start(tt_dram[i][:], tt_st[:])

                    with tc.tile_pool(name="vhi", bufs=1) as vhi_pool:
                        v_hi = vhi_pool.tile([P, NV // 2, D], DT)
                        for so in range(NV // 2):
                            nc.sync.dma_start(v_hi[:, so], v_r[:, NV // 2 + so])
                        for i in ORDER_A:
                            attention_block(i)
                    # v_hi freed: prefetch phase-4 inputs under the tail blocks
                    with tc.tile_pool(name="wvpool", bufs=1) as wvpool, \
                         tc.tile_pool(name="dwork", bufs=3) as dwork, \
                         tc.tile_pool(name="owork", bufs=2) as owork:
                        wv_sb = wvpool.tile([P, KO, D], DT)
                        for ko in range(KO):
                            nc.sync.dma_start(wv_sb[:, ko], wv_r[:, ko])
                        tt_rds = {}
                        for i in ORDER_A[:3]:
                            tt_rds[i] = dwork.tile([P, KO, P], DT, tag="ttrd",
                                                   name=f"ttrd_{i}")
                            nc.sync.dma_start(tt_rds[i][:], tt_dram[i][:])
                        for i in ORDER_B:
                            attention_block(i)

                        # ---- Phase 4: out = TT.T @ Wv (pure matmuls) ----
                        for step in range(NBLK):
                            i = (ORDER_A + ORDER_B)[step]
                            if i in tt_rds:
                                tt_rd = tt_rds.pop(i)
                            else:
                                tt_rd = dwork.tile([P, KO, P], DT, tag="ttrd",
                                                   name=f"ttrd_{i}")
                                nc.sync.dma_start(tt_rd[:], tt_dram[i][:])
                            if step % 2 == 0:
                                ps_o0 = ps_t.tile([P, CH], F32, tag="T0",
                                                  name=f"o0_{i}")
                                ps_o1 = ps_t.tile([P, CH], F32, tag="T1",
                                                  name=f"o1_{i}")
                            else:
                                ps_o0 = ps_tr.tile([P, CH], F32, tag="tr",
                                                   name=f"o0_{i}")
                                ps_o1 = ps_tr.tile([P, CH], F32, tag="tr",
                                                   name=f"o1_{i}")
                            for d in range(KO):
                                nc.tensor.matmul(
                                    ps_o0[:], tt_rd[:, d], wv_sb[:, d, 0:CH],
                                    start=(d == 0), stop=(d == KO - 1))
                                nc.tensor.matmul(
                                    ps_o1[:], tt_rd[:, d], wv_sb[:, d, CH:D],
                                    start=(d == 0), stop=(d == KO - 1))
                            o_sb = owork.tile([P, D], F32, tag="osb")
                            nc.vector.tensor_copy(o_sb[:, 0:CH], ps_o0[:])
                            nc.vector.tensor_copy(o_sb[:, CH:D], ps_o1[:])
                            nc.sync.dma_start(out[bass.ts(i, P), :], o_sb[:])

            _vstack.close()

    nc.compile()
    _cached["nc"] = nc
    return nc


LAST_RESULT = None


def kernel(q, k, v, Wq, Wk, Wv, mask):
    global LAST_RESULT
    q = np.asarray(q, dtype=np.float32)
    k = np.asarray(k, dtype=np.float32)
    v = np.asarray(v, dtype=np.float32)
    Wq = np.asarray(Wq, dtype=np.float32)
    Wk = np.asarray(Wk, dtype=np.float32)
    Wv = np.asarray(Wv, dtype=np.float32)

    nc = _build()

    wm = np.ascontiguousarray(
        (Wq.astype(np.float64) @ Wk.astype(np.float64).T
         / np.sqrt(np.float64(D))).astype(np.float32))
    wv_c = np.ascontiguousarray(Wv)
    ident = np.eye(P, dtype=np.float32)

    masks = []
    r = np.arange(P)[:, None]
    c = np.arange(CH)[None, :]
    for h in range(2):
        m = np.zeros((P, NBLK, CH), dtype=np.float32)
        for i in range(NBLK):
            j = BLOCKS[h][i]
            q0 = P * j
            nch = (W[i] + CH - 1) // CH
            last_off = CH * (nch - 1)
            w_last = W[i] - last_off
            mi = np.where(last_off + c <= q0 + r, 0.0, NEG)
            mi[:, w_last:] = 0.0
            m[:, i, :] = mi
        masks.append(m.astype(ml_dtypes.bfloat16))

    in_maps = []
    for core in range(8):
        b, h = core // 2, core % 2
        blocks = BLOCKS[h]
        qTb = q[b].T  # [D, S]
        cols = np.concatenate([np.arange(j * P, (j + 1) * P) for j in blocks])
        in_maps.append({
            "qT": np.ascontiguousarray(qTb[:, cols]),
            "kT": np.ascontiguousarray(k[b].T),
            "v": np.ascontiguousarray(v[b]),
            "wq": wm, "wv": wv_c,
            "mask": masks[h], "ident": ident,
        })

    res = run_bass_kernel_spmd(nc, in_maps, list(range(8)),
                               trace=bool(os.environ.get("KERNEL_TRACE")))
    LAST_RESULT = res

    out = np.empty((B, S, D), dtype=np.float32)
    for core in range(8):
        b, h = core // 2, core % 2
        oc = np.asarray(res.results[core]["out"], dtype=np.float32)
        for pos, j in enumerate(BLOCKS[h]):
            out[b, j * P:(j + 1) * P, :] = oc[pos * P:(pos + 1) * P, :]
    return out
